# revision 32
# baseline (speedup 1.0000x reference)
"""Trainium2 Bass kernel for nn_Contrastive_Loss (bs=8192, hidden=2048, 8 cores).

Math: reference(X, Y) = cl(X,Y) + cl(Y,X) where
  cl(A,B)[i] = -log(E_ii / (colsum_i(E) - E_ii)),  E = exp(norm(A) @ norm(B).T)
Since norm(Y)@norm(X).T = S.T, the second term's column sums are the first
term's row sums and the diagonals coincide.  With E = exp(S):
  out[i] = log(rowsum_i(E) - E_ii) + log(colsum_i(E) - E_ii) - 2*S_ii

Sharding (v4): core r holds X rows [1024r, 1024(r+1)) and computes the
transposed block E^T[all 8192 j, its 1024 i] = exp(S_ij).  Host supplies
X^T (bf16), X/Y row shards (fp8) and the full raw Y^T (fp8) - layout/dtype
staging only.

Per-emit structure (software-pipelined across `repeat` emits so the
marginal cost is just the PE stream):
 - P: XT/XS/YS loads; x*y diag products on Pool; row norms as DVE
   mul+reduce into one [128,16] tile (y|x halves).
 - B: 64 j-tiles; per tile 16 DoubleRow fp8 matmuls (k2 outer, halves
   inner to share the stationary), ONE [128,1024] exp on ACT with
   per-partition 1/||y_j|| scale, fp32 accum_out = colsum partials,
   fp8 output in pair slots; per pair 2 DoubleRow ones-matmuls
   accumulate row sums in PSUM across the phase.  The NEXT emit's
   1/norm chain (single Ln+Exp), invx broadcast (Pool
   partition_broadcast), X^T fold, fp8 casts and AllGather are hooked
   into the middle of this stream so they hide under the matmuls.
 - T: colsum ReduceScatter + finale, emitted after the following B so
   its semaphore waits never block the next emit's issue streams.
"""

import numpy as np
import ml_dtypes
from contextlib import ExitStack

import concourse.bass as bass
import concourse.bacc as bacc
import concourse.mybir as mybir
import concourse.tile as tile
from concourse.bass_utils import run_bass_kernel_spmd

FP32 = mybir.dt.float32
BF16 = mybir.dt.bfloat16
FP8 = mybir.dt.float8e4

BS = 8192      # batch (rows of X and Y)
H = 2048       # hidden
NCORES = 8
RPC = BS // NCORES   # rows per core = 1024

AF = mybir.ActivationFunctionType
ALU = mybir.AluOpType
DR = mybir.MatmulPerfMode.DoubleRow


def build(bs=BS, h=H, ncores=NCORES, repeat=1, no_coll=False, HOOK0=40):
    """repeat>1 emits the computation R times back-to-back in one NEFF --
    used by the timing harness to difference away fixed dispatch overhead.
    kernel() always uses repeat=1.  no_coll=True replaces the collectives
    with local DMA copies (timing experiments ONLY -- results are wrong)."""
    rpc = bs // ncores
    kt_n = h // 128          # contraction tiles = 16
    jt_n = bs // 128         # j (Y-row) tiles per core = 64
    mb_n = rpc // 128        # own-row 128-blocks = 8
    gs = 8                   # j-tiles per YT load group
    ng = jt_n // gs          # load groups = 8
    groups = [list(range(ncores))]

    nc = bacc.Bacc("TRN2", target_bir_lowering=False, num_devices=ncores)
    XT = nc.dram_tensor("XT", [h, rpc], BF16, kind="ExternalInput")
    XS = nc.dram_tensor("XS", [rpc, h], FP8, kind="ExternalInput")
    YS = nc.dram_tensor("YS", [rpc, h], FP8, kind="ExternalInput")
    YT = nc.dram_tensor("YT", [h, bs], FP8, kind="ExternalInput")
    OUT = nc.dram_tensor("OUT", [rpc, 1], FP32, kind="ExternalOutput")
    YTR = YT.rearrange("(k p) n -> p k n", p=128)

    with tile.TileContext(nc) as tc, ExitStack() as ctx:
        dram = ctx.enter_context(tc.tile_pool(name="dram", bufs=2, space="DRAM"))
        xsp = ctx.enter_context(tc.tile_pool(name="xsp", bufs=4))
        ysp = ctx.enter_context(tc.tile_pool(name="ysp", bufs=4))
        junkv = ctx.enter_context(tc.tile_pool(name="junkv", bufs=2))
        junkd = ctx.enter_context(tc.tile_pool(name="junkd", bufs=8))
        xpool = ctx.enter_context(tc.tile_pool(name="xpool", bufs=1))
        x8pool = ctx.enter_context(tc.tile_pool(name="x8pool", bufs=2))
        ypool = ctx.enter_context(tc.tile_pool(name="ypool", bufs=3))
        e2pool = ctx.enter_context(tc.tile_pool(name="e2pool", bufs=2))
        stat = ctx.enter_context(tc.tile_pool(name="stat", bufs=2))
        brd = ctx.enter_context(tc.tile_pool(name="brd", bufs=2))
        consts = ctx.enter_context(tc.tile_pool(name="consts", bufs=1))
        stpsum = ctx.enter_context(tc.tile_pool(name="stpsum", bufs=3, space="PSUM"))
        rpsum = ctx.enter_context(tc.tile_pool(name="rpsum", bufs=1, space="PSUM"))

        ones2 = consts.tile([128, 2, 32], FP8)
        nc.vector.memset(ones2[:], 1.0)

        def alloc_state():
            s = {}
            # [:, :8]=y, [:, 8:]=x
            s["ssqb"] = stat.tile([128, 2 * mb_n], FP32, tag="ssqb", name="ssqb")
            s["invb"] = stat.tile([128, 2 * mb_n], FP32, tag="invb", name="invb")
            s["invy_all"] = stat.tile([128, jt_n], FP32, tag="iya", name="iya")
            s["csacc"] = stat.tile([128, jt_n], FP32, tag="csacc", name="csacc")
            s["sdraw"] = stat.tile([128, mb_n], FP32, tag="sdraw", name="sdraw")
            s["sdiag"] = stat.tile([128, mb_n], FP32, tag="sdiag", name="sdiag")
            s["invx_row"] = brd.tile([1, rpc], FP32, tag="ivr", name="ivr")
            s["invxb"] = brd.tile([128, rpc], FP32, tag="ivb", name="ivb")
            s["INVXD"] = dram.tile([rpc], FP32, name="INVXD", tag="INVXD")
            s["INVYL"] = dram.tile([rpc], FP32, name="INVYL", tag="INVYL")
            s["INVYA"] = dram.tile([bs], FP32, addr_space="Shared",
                                   name="INVYA", tag="INVYA")
            s["CS"] = dram.tile([bs], FP32, name="CSD", tag="CSD")
            s["CSR"] = dram.tile([rpc], FP32, name="CSRD", tag="CSRD")
            s["RSD"] = dram.tile([rpc], FP32, name="RSD", tag="RSD")
            s["rps"] = [rpsum.tile([32, 512], FP32, tag=f"rps{i}", name=f"rps{i}")
                        for i in range(2)]
            return s

        def p_loads_norms(s):
            """Loads + DVE norms + Pool diag products for one emit.  For
            pipelined emits this lands before the previous B so it all
            executes under that B's matmul stream."""
            xnt = xpool.tile([128, kt_n, rpc], BF16, tag="xnt", name="xnt")
            nc.scalar.dma_start(
                out=xnt[:], in_=XT.rearrange("(k p) m -> p k m", p=128)
            )
            s["xnt"] = xnt
            xss, yss, jds = [], [], []
            for m in range(mb_n):
                r0 = m * 128
                xs = xsp.tile([128, h], FP8, tag="ldx", name="ldx")
                nc.gpsimd.dma_start(out=xs[:], in_=XS[r0 : r0 + 128, :])
                xss.append(xs)
            for m in range(mb_n):
                r0 = m * 128
                ys = ysp.tile([128, h], FP8, tag="ldy", name="ldy")
                nc.gpsimd.dma_start(out=ys[:], in_=YS[r0 : r0 + 128, :])
                yss.append(ys)
            # diag products x*y on Pool (reduced on DVE later, off-path)
            for m in range(mb_n):
                jd = junkd.tile([128, h], BF16, tag="junkd", name="junkd")
                nc.gpsimd.tensor_mul(jd[:], xss[m][:], yss[m][:])
                jds.append(jd)
            s["jds"] = jds
            ssqb = s["ssqb"]
            for m in range(mb_n):
                jx = junkv.tile([128, h], BF16, tag="junkv", name="junkv")
                nc.vector.tensor_mul(jx[:], xss[m][:], xss[m][:])
                nc.vector.reduce_sum(
                    ssqb[:, mb_n + m : mb_n + m + 1], jx[:], mybir.AxisListType.X
                )
            for m in range(mb_n):
                jy = junkv.tile([128, h], BF16, tag="junkv", name="junkv")
                nc.vector.tensor_mul(jy[:], yss[m][:], yss[m][:])
                nc.vector.reduce_sum(
                    ssqb[:, m : m + 1], jy[:], mybir.AxisListType.X
                )
            nc.vector.tensor_scalar_max(ssqb[:], ssqb[:], 1e-16)

        def p_inv(s):
            """1/norms = exp(-0.5 ln(ssq)) (one ACT pair), AllGather of
            1/||y||, invx relayout + partition broadcast, X^T fold.  For
            pipelined emits these are re-emitted by the B hooks instead
            so they interleave with the exp stream."""
            p_inv_act(s)
            p_inv_pool(s)
            p_fold(s)

        def p_inv_act(s):
            nc.scalar.activation(s["invb"][:], s["ssqb"][:], AF.Ln)
            nc.scalar.activation(s["invb"][:], s["invb"][:], AF.Exp, scale=-0.5)

        def p_inv_pool(s):
            invy_own = s["invb"][:, 0:mb_n]
            nc.gpsimd.dma_start(
                out=s["INVYL"].rearrange("(a b) -> b a", b=128), in_=invy_own
            )
            if no_coll:
                nc.gpsimd.dma_start(out=s["INVYA"][0:rpc], in_=s["INVYL"][:])
            else:
                nc.gpsimd.collective_compute(
                    "AllGather", ALU.bypass, replica_groups=groups,
                    ins=[s["INVYL"].opt()], outs=[s["INVYA"].opt()],
                )
            nc.gpsimd.dma_start(
                out=s["invy_all"][:], in_=s["INVYA"].rearrange("(a b) -> b a", b=128)
            )
            nc.gpsimd.dma_start(
                out=s["INVXD"].rearrange("(a b) -> b a", b=128),
                in_=s["invb"][:, mb_n : 2 * mb_n],
            )
            nc.gpsimd.dma_start(out=s["invx_row"][:], in_=s["INVXD"][:])
            nc.gpsimd.partition_broadcast(s["invxb"][:], s["invx_row"][:])

        def p_fold(s):
            # Pool tensor ops write fp8 exactly (DVE's fp8 writes are
            # broken on this silicon), so the fold produces the fp8
            # moving operand directly - no ACT cast pass
            xnt, invxb = s["xnt"], s["invxb"]
            xnt8 = x8pool.tile([128, kt_n, rpc], FP8, tag="x8", name="x8")
            for kt in range(kt_n):
                nc.gpsimd.tensor_mul(xnt8[:, kt, :], xnt[:, kt, :], invxb[:])
            s["xnt8"] = xnt8

        def p_diag(s):
            for m in range(mb_n):
                nc.vector.reduce_sum(
                    s["sdraw"][:, m : m + 1], s["jds"][m][:], mybir.AxisListType.X
                )
            nc.vector.tensor_mul(
                s["sdiag"][:], s["sdraw"][:], s["invb"][:, mb_n : 2 * mb_n]
            )
            nc.vector.tensor_mul(
                s["sdiag"][:], s["sdiag"][:], s["invb"][:, 0:mb_n]
            )

        def b_phase(s, nxt):
            """The PE stream: 64 j-tiles.  nxt's prologue compute is hooked
            at fixed jt positions so it hides under this emit's matmuls."""
            xnt8, rps = s["xnt8"], s["rps"]
            e2 = None
            for g in range(ng):
                ytg = ypool.tile([128, kt_n, 1024], FP8, tag="yt", name="yt")
                nc.sync.dma_start(
                    out=ytg[:], in_=YTR[:, :, g * 1024 : g * 1024 + 1024]
                )
                for jl in range(gs):
                    jt = g * gs + jl
                    st = stpsum.tile([128, 1024], FP32, tag="st", name="st")
                    for k2 in range(kt_n // 2):
                        for hh in range(2):
                            nc.tensor.matmul(
                                st[:, hh * 512 : hh * 512 + 512],
                                lhsT=ytg[:, 2 * k2 : 2 * k2 + 2,
                                         jl * 128 : jl * 128 + 128],
                                rhs=xnt8[:, 2 * k2 : 2 * k2 + 2,
                                         hh * 512 : hh * 512 + 512],
                                start=(k2 == 0), stop=(k2 == kt_n // 2 - 1),
                                perf_mode=DR,
                            )
                    if jt % 2 == 0:
                        e2 = e2pool.tile([128, 2, 1024], FP8, tag="E", name="E")
                    nc.scalar.activation(
                        e2[:, jt % 2, :], st[:], AF.Exp,
                        scale=s["invy_all"][:, jt : jt + 1],
                        accum_out=s["csacc"][:, jt : jt + 1],
                    )
                    if jt % 2 == 1:
                        pair = jt // 2
                        for hh in range(2):
                            nc.tensor.matmul(
                                rps[hh][:], lhsT=ones2[:],
                                rhs=e2[:, :, hh * 512 : hh * 512 + 512],
                                start=(pair == 0), stop=(pair == jt_n // 2 - 1),
                                perf_mode=DR,
                            )
                    if nxt is not None:
                        # hooks sit late enough that the next emit's DVE
                        # norms (ready ~95us into this phase) never block
                        # the in-order ACT exp stream
                        if jt == HOOK0:
                            p_inv_act(nxt)
                            p_inv_pool(nxt)
                        elif jt == HOOK0 + 2:
                            p_fold(nxt)
                        elif jt == HOOK0 + 6:
                            p_diag(nxt)

        def t_early(s):
            nc.gpsimd.dma_start(
                out=s["CS"].rearrange("(a b) -> b a", b=128), in_=s["csacc"][:]
            )
            if no_coll:
                nc.gpsimd.dma_start(out=s["CSR"][:], in_=s["CS"][0:rpc])
            else:
                nc.gpsimd.collective_compute(
                    "ReduceScatter", ALU.add, replica_groups=groups,
                    ins=[s["CS"].opt()], outs=[s["CSR"].opt()],
                )
            rs_row = stat.tile([1, rpc], FP32, tag="rsrow", name="rsrow")
            for hh in range(2):
                nc.vector.tensor_copy(
                    rs_row[:, hh * 512 : hh * 512 + 512], s["rps"][hh][0:1, :]
                )
            nc.gpsimd.dma_start(out=s["RSD"][:], in_=rs_row[:])
            rsum = stat.tile([128, mb_n], FP32, tag="rsum", name="rsum")
            nc.gpsimd.dma_start(
                out=rsum[:], in_=s["RSD"].rearrange("(a b) -> b a", b=128)
            )
            csr = stat.tile([128, mb_n], FP32, tag="csr", name="csr")
            nc.gpsimd.dma_start(
                out=csr[:], in_=s["CSR"].rearrange("(a b) -> b a", b=128)
            )
            s["rsum"], s["csr"] = rsum, csr

        def t_late(s):
            edig = stat.tile([128, mb_n], FP32, tag="edig", name="edig")
            nc.scalar.activation(edig[:], s["sdiag"][:], AF.Exp)
            negb = stat.tile([128, 2 * mb_n], FP32, tag="negb", name="negb")
            nc.vector.tensor_sub(negb[:, 0:mb_n], s["rsum"][:], edig[:])
            nc.vector.tensor_sub(negb[:, mb_n : 2 * mb_n], s["csr"][:], edig[:])
            lnb = stat.tile([128, 2 * mb_n], FP32, tag="lnb", name="lnb")
            nc.scalar.activation(lnb[:], negb[:], AF.Ln)
            res = stat.tile([128, mb_n], FP32, tag="res", name="res")
            nc.vector.tensor_add(res[:], lnb[:, 0:mb_n], lnb[:, mb_n : 2 * mb_n])
            d2 = stat.tile([128, mb_n], FP32, tag="d2", name="d2")
            nc.vector.tensor_scalar_mul(d2[:], s["sdiag"][:], -2.0)
            nc.vector.tensor_add(res[:], res[:], d2[:])
            nc.gpsimd.dma_start(
                out=OUT.rearrange("(a b) c -> b (a c)", b=128), in_=res[:]
            )

        # ---- pipelined driver ----
        states = {0: alloc_state()}
        p_loads_norms(states[0])
        p_inv(states[0])
        p_diag(states[0])
        for r in range(repeat):
            if r + 1 < repeat:
                states[r + 1] = alloc_state()
                p_loads_norms(states[r + 1])
            if r > 0:
                t_early(states[r - 1])
            b_phase(states[r], states.get(r + 1))
            if r > 0:
                t_late(states[r - 1])
                del states[r - 1]
        t_early(states[repeat - 1])
        t_late(states[repeat - 1])

    nc.compile()
    return nc


_CACHE = {}


def _get_nc():
    if "nc" not in _CACHE:
        _CACHE["nc"] = build()
    return _CACHE["nc"]


def make_in_maps(X, Y, ncores=NCORES, rpc=RPC):
    bf16 = ml_dtypes.bfloat16
    fp8 = ml_dtypes.float8_e4m3
    yt = np.ascontiguousarray(Y.T.astype(fp8))
    maps = []
    for i in range(ncores):
        xs = X[i * rpc : (i + 1) * rpc]
        maps.append({
            "XT": np.ascontiguousarray(xs.T.astype(bf16)),
            "XS": np.ascontiguousarray(xs.astype(fp8)),
            "YS": np.ascontiguousarray(Y[i * rpc : (i + 1) * rpc].astype(fp8)),
            "YT": yt,
        })
    return maps


def kernel(X, Y):
    X = np.ascontiguousarray(np.asarray(X, dtype=np.float32))
    Y = np.ascontiguousarray(np.asarray(Y, dtype=np.float32))
    assert X.shape == (BS, H) and Y.shape == (BS, H)
    nc = _get_nc()
    r = run_bass_kernel_spmd(nc, make_in_maps(X, Y), list(range(NCORES)))
    out = np.concatenate([r.results[i]["OUT"] for i in range(NCORES)], axis=0)
    return out.astype(np.float32)


# revision 33
# speedup vs baseline: 1.0403x; 1.0403x over previous
"""Trainium2 Bass kernel for nn_Contrastive_Loss (bs=8192, hidden=2048, 8 cores).

Math: reference(X, Y) = cl(X,Y) + cl(Y,X) where
  cl(A,B)[i] = -log(E_ii / (colsum_i(E) - E_ii)),  E = exp(norm(A) @ norm(B).T)
Since norm(Y)@norm(X).T = S.T, the second term's column sums are the first
term's row sums and the diagonals coincide.  With E = exp(S):
  out[i] = log(rowsum_i(E) - E_ii) + log(colsum_i(E) - E_ii) - 2*S_ii

Sharding (v4): core r holds X rows [1024r, 1024(r+1)) and computes the
transposed block E^T[all 8192 j, its 1024 i] = exp(S_ij).  Host supplies
X^T (bf16), X/Y row shards (fp8) and the full raw Y^T (fp8) - layout/dtype
staging only.

Per-emit structure (software-pipelined across `repeat` emits so the
marginal cost is just the PE stream):
 - P: XT/XS/YS loads; x*y diag products on Pool; row norms as DVE
   mul+reduce into one [128,16] tile (y|x halves).
 - B: 64 j-tiles; per tile 16 DoubleRow fp8 matmuls (k2 outer, halves
   inner to share the stationary), ONE [128,1024] exp on ACT with
   per-partition 1/||y_j|| scale, fp32 accum_out = colsum partials,
   fp8 output in pair slots; per pair 2 DoubleRow ones-matmuls
   accumulate row sums in PSUM across the phase.  The NEXT emit's
   1/norm chain (single Ln+Exp), invx broadcast (Pool
   partition_broadcast), X^T fold, fp8 casts and AllGather are hooked
   into the middle of this stream so they hide under the matmuls.
 - T: colsum ReduceScatter + finale, emitted after the following B so
   its semaphore waits never block the next emit's issue streams.
"""

import numpy as np
import ml_dtypes
from contextlib import ExitStack

import concourse.bass as bass
import concourse.bacc as bacc
import concourse.mybir as mybir
import concourse.tile as tile
from concourse.bass_utils import run_bass_kernel_spmd

FP32 = mybir.dt.float32
BF16 = mybir.dt.bfloat16
FP8 = mybir.dt.float8e4

BS = 8192      # batch (rows of X and Y)
H = 2048       # hidden
NCORES = 8
RPC = BS // NCORES   # rows per core = 1024

AF = mybir.ActivationFunctionType
ALU = mybir.AluOpType
DR = mybir.MatmulPerfMode.DoubleRow


def build(bs=BS, h=H, ncores=NCORES, repeat=1, no_coll=False, HOOK0=40):
    """repeat>1 emits the computation R times back-to-back in one NEFF --
    used by the timing harness to difference away fixed dispatch overhead.
    kernel() always uses repeat=1.  no_coll=True replaces the collectives
    with local DMA copies (timing experiments ONLY -- results are wrong)."""
    rpc = bs // ncores
    kt_n = h // 128          # contraction tiles = 16
    jt_n = bs // 128         # j (Y-row) tiles per core = 64
    mb_n = rpc // 128        # own-row 128-blocks = 8
    gs = 8                   # j-tiles per YT load group
    ng = jt_n // gs          # load groups = 8
    groups = [list(range(ncores))]

    nc = bacc.Bacc("TRN2", target_bir_lowering=False, num_devices=ncores)
    XT = nc.dram_tensor("XT", [h, rpc], BF16, kind="ExternalInput")
    XS = nc.dram_tensor("XS", [rpc, h], FP8, kind="ExternalInput")
    YS = nc.dram_tensor("YS", [rpc, h], FP8, kind="ExternalInput")
    YT = nc.dram_tensor("YT", [h, bs], FP8, kind="ExternalInput")
    OUT = nc.dram_tensor("OUT", [rpc, 1], FP32, kind="ExternalOutput")
    YTR = YT.rearrange("(k p) n -> p k n", p=128)

    with tile.TileContext(nc) as tc, ExitStack() as ctx:
        dram = ctx.enter_context(tc.tile_pool(name="dram", bufs=2, space="DRAM"))
        xsp = ctx.enter_context(tc.tile_pool(name="xsp", bufs=4))
        ysp = ctx.enter_context(tc.tile_pool(name="ysp", bufs=4))
        junkv = ctx.enter_context(tc.tile_pool(name="junkv", bufs=2))
        junkd = ctx.enter_context(tc.tile_pool(name="junkd", bufs=8))
        xpool = ctx.enter_context(tc.tile_pool(name="xpool", bufs=1))
        x8pool = ctx.enter_context(tc.tile_pool(name="x8pool", bufs=2))
        ypool = ctx.enter_context(tc.tile_pool(name="ypool", bufs=3))
        e2pool = ctx.enter_context(tc.tile_pool(name="e2pool", bufs=2))
        stat = ctx.enter_context(tc.tile_pool(name="stat", bufs=2))
        brd = ctx.enter_context(tc.tile_pool(name="brd", bufs=2))
        consts = ctx.enter_context(tc.tile_pool(name="consts", bufs=1))
        stpsum = ctx.enter_context(tc.tile_pool(name="stpsum", bufs=3, space="PSUM"))
        rpsum = ctx.enter_context(tc.tile_pool(name="rpsum", bufs=1, space="PSUM"))

        ones2 = consts.tile([128, 2, 32], FP8)
        nc.vector.memset(ones2[:], 1.0)

        def alloc_state():
            s = {}
            # [:, :8]=y, [:, 8:]=x
            s["ssqb"] = stat.tile([128, 2 * mb_n], FP32, tag="ssqb", name="ssqb")
            s["invb"] = stat.tile([128, 2 * mb_n], FP32, tag="invb", name="invb")
            s["invy_all"] = stat.tile([128, jt_n], FP32, tag="iya", name="iya")
            s["csacc"] = stat.tile([128, jt_n], FP32, tag="csacc", name="csacc")
            s["sdraw"] = stat.tile([128, mb_n], FP32, tag="sdraw", name="sdraw")
            s["sdiag"] = stat.tile([128, mb_n], FP32, tag="sdiag", name="sdiag")
            s["invx_row"] = brd.tile([1, rpc], FP32, tag="ivr", name="ivr")
            s["invxb"] = brd.tile([128, rpc], FP32, tag="ivb", name="ivb")
            s["INVXD"] = dram.tile([rpc], FP32, name="INVXD", tag="INVXD")
            s["INVYL"] = dram.tile([rpc], FP32, name="INVYL", tag="INVYL")
            s["INVYA"] = dram.tile([bs], FP32, addr_space="Shared",
                                   name="INVYA", tag="INVYA")
            s["CS"] = dram.tile([bs], FP32, name="CSD", tag="CSD")
            s["CSR"] = dram.tile([rpc], FP32, name="CSRD", tag="CSRD")
            s["RSD"] = dram.tile([rpc], FP32, name="RSD", tag="RSD")
            s["rps"] = [rpsum.tile([32, 512], FP32, tag=f"rps{i}", name=f"rps{i}")
                        for i in range(2)]
            return s

        def p_loads_norms(s):
            """Loads + DVE norms + Pool diag products for one emit.  For
            pipelined emits this lands before the previous B so it all
            executes under that B's matmul stream."""
            xnt = xpool.tile([128, kt_n, rpc], BF16, tag="xnt", name="xnt")
            nc.scalar.dma_start(
                out=xnt[:], in_=XT.rearrange("(k p) m -> p k m", p=128)
            )
            s["xnt"] = xnt
            xss, yss, jds = [], [], []
            for m in range(mb_n):
                r0 = m * 128
                xs = xsp.tile([128, h], FP8, tag="ldx", name="ldx")
                nc.gpsimd.dma_start(out=xs[:], in_=XS[r0 : r0 + 128, :])
                xss.append(xs)
            for m in range(mb_n):
                r0 = m * 128
                ys = ysp.tile([128, h], FP8, tag="ldy", name="ldy")
                nc.gpsimd.dma_start(out=ys[:], in_=YS[r0 : r0 + 128, :])
                yss.append(ys)
            # diag products x*y on Pool (reduced on DVE later, off-path)
            for m in range(mb_n):
                jd = junkd.tile([128, h], BF16, tag="junkd", name="junkd")
                nc.gpsimd.tensor_mul(jd[:], xss[m][:], yss[m][:])
                jds.append(jd)
            s["jds"] = jds
            ssqb = s["ssqb"]
            for m in range(mb_n):
                jx = junkv.tile([128, h], BF16, tag="junkv", name="junkv")
                nc.vector.tensor_mul(jx[:], xss[m][:], xss[m][:])
                nc.vector.reduce_sum(
                    ssqb[:, mb_n + m : mb_n + m + 1], jx[:], mybir.AxisListType.X
                )
            for m in range(mb_n):
                jy = junkv.tile([128, h], BF16, tag="junkv", name="junkv")
                nc.vector.tensor_mul(jy[:], yss[m][:], yss[m][:])
                nc.vector.reduce_sum(
                    ssqb[:, m : m + 1], jy[:], mybir.AxisListType.X
                )
            nc.vector.tensor_scalar_max(ssqb[:], ssqb[:], 1e-16)

        def p_inv(s):
            """1/norms = exp(-0.5 ln(ssq)) (one ACT pair), AllGather of
            1/||y||, invx relayout + partition broadcast, X^T fold.  For
            pipelined emits these are re-emitted by the B hooks instead
            so they interleave with the exp stream."""
            p_inv_act(s)
            p_inv_pool(s)
            p_fold(s)

        def p_inv_act(s):
            nc.scalar.activation(s["invb"][:], s["ssqb"][:], AF.Ln)
            nc.scalar.activation(s["invb"][:], s["invb"][:], AF.Exp, scale=-0.5)

        def p_inv_pool(s):
            invy_own = s["invb"][:, 0:mb_n]
            nc.gpsimd.dma_start(
                out=s["INVYL"].rearrange("(a b) -> b a", b=128), in_=invy_own
            )
            if no_coll:
                nc.gpsimd.dma_start(out=s["INVYA"][0:rpc], in_=s["INVYL"][:])
            else:
                nc.gpsimd.collective_compute(
                    "AllGather", ALU.bypass, replica_groups=groups,
                    ins=[s["INVYL"].opt()], outs=[s["INVYA"].opt()],
                )
            nc.gpsimd.dma_start(
                out=s["invy_all"][:], in_=s["INVYA"].rearrange("(a b) -> b a", b=128)
            )
            nc.gpsimd.dma_start(
                out=s["INVXD"].rearrange("(a b) -> b a", b=128),
                in_=s["invb"][:, mb_n : 2 * mb_n],
            )
            nc.gpsimd.dma_start(out=s["invx_row"][:], in_=s["INVXD"][:])
            nc.gpsimd.partition_broadcast(s["invxb"][:], s["invx_row"][:])

        def p_fold(s):
            # Pool tensor ops write fp8 exactly (DVE's fp8 writes are
            # broken on this silicon), so the fold produces the fp8
            # moving operand directly - no ACT cast pass
            xnt, invxb = s["xnt"], s["invxb"]
            xnt8 = x8pool.tile([128, kt_n, rpc], FP8, tag="x8", name="x8")
            for kt in range(kt_n):
                nc.gpsimd.tensor_mul(xnt8[:, kt, :], xnt[:, kt, :], invxb[:])
            s["xnt8"] = xnt8

        def p_diag(s):
            for m in range(mb_n):
                nc.vector.reduce_sum(
                    s["sdraw"][:, m : m + 1], s["jds"][m][:], mybir.AxisListType.X
                )
            nc.vector.tensor_mul(
                s["sdiag"][:], s["sdraw"][:], s["invb"][:, mb_n : 2 * mb_n]
            )
            nc.vector.tensor_mul(
                s["sdiag"][:], s["sdiag"][:], s["invb"][:, 0:mb_n]
            )

        def b_phase(s, nxt):
            """The PE stream: 64 j-tiles.  nxt's prologue compute is hooked
            at fixed jt positions so it hides under this emit's matmuls."""
            xnt8, rps = s["xnt8"], s["rps"]
            e2 = None
            pending = None
            for g in range(ng):
                ytg = ypool.tile([128, kt_n, 1024], FP8, tag="yt", name="yt")
                nc.sync.dma_start(
                    out=ytg[:], in_=YTR[:, :, g * 1024 : g * 1024 + 1024]
                )
                for jl in range(gs):
                    jt = g * gs + jl
                    st = stpsum.tile([128, 1024], FP32, tag="st", name="st")
                    for k2 in range(kt_n // 2):
                        for hh in range(2):
                            nc.tensor.matmul(
                                st[:, hh * 512 : hh * 512 + 512],
                                lhsT=ytg[:, 2 * k2 : 2 * k2 + 2,
                                         jl * 128 : jl * 128 + 128],
                                rhs=xnt8[:, 2 * k2 : 2 * k2 + 2,
                                         hh * 512 : hh * 512 + 512],
                                start=(k2 == 0), stop=(k2 == kt_n // 2 - 1),
                                perf_mode=DR,
                            )
                    # rowsum pair-matmul deferred by one extra chain so
                    # the pair's second exp has ~3.4us of slack before PE
                    # needs its fp8 output
                    if pending is not None:
                        pair, pe2 = pending
                        pending = None
                        for hh in range(2):
                            nc.tensor.matmul(
                                rps[hh][:], lhsT=ones2[:],
                                rhs=pe2[:, :, hh * 512 : hh * 512 + 512],
                                start=(pair == 0), stop=False,
                                perf_mode=DR,
                            )
                    if jt % 2 == 0:
                        e2 = e2pool.tile([128, 2, 1024], FP8, tag="E", name="E")
                    nc.scalar.activation(
                        e2[:, jt % 2, :], st[:], AF.Exp,
                        scale=s["invy_all"][:, jt : jt + 1],
                        accum_out=s["csacc"][:, jt : jt + 1],
                    )
                    if jt % 2 == 1:
                        pending = (jt // 2, e2)
                    if nxt is not None:
                        # hooks sit late enough that the next emit's DVE
                        # norms (ready ~95us into this phase) never block
                        # the in-order ACT exp stream
                        if jt == HOOK0:
                            p_inv_act(nxt)
                            p_inv_pool(nxt)
                        elif jt == HOOK0 + 2:
                            p_fold(nxt)
                        elif jt == HOOK0 + 6:
                            p_diag(nxt)
            # flush the last pair, closing the accumulation groups
            pair, pe2 = pending
            for hh in range(2):
                nc.tensor.matmul(
                    rps[hh][:], lhsT=ones2[:],
                    rhs=pe2[:, :, hh * 512 : hh * 512 + 512],
                    start=(pair == 0), stop=True,
                    perf_mode=DR,
                )

        def t_early(s):
            nc.gpsimd.dma_start(
                out=s["CS"].rearrange("(a b) -> b a", b=128), in_=s["csacc"][:]
            )
            if no_coll:
                nc.gpsimd.dma_start(out=s["CSR"][:], in_=s["CS"][0:rpc])
            else:
                nc.gpsimd.collective_compute(
                    "ReduceScatter", ALU.add, replica_groups=groups,
                    ins=[s["CS"].opt()], outs=[s["CSR"].opt()],
                )
            rs_row = stat.tile([1, rpc], FP32, tag="rsrow", name="rsrow")
            for hh in range(2):
                nc.vector.tensor_copy(
                    rs_row[:, hh * 512 : hh * 512 + 512], s["rps"][hh][0:1, :]
                )
            nc.gpsimd.dma_start(out=s["RSD"][:], in_=rs_row[:])
            rsum = stat.tile([128, mb_n], FP32, tag="rsum", name="rsum")
            nc.gpsimd.dma_start(
                out=rsum[:], in_=s["RSD"].rearrange("(a b) -> b a", b=128)
            )
            csr = stat.tile([128, mb_n], FP32, tag="csr", name="csr")
            nc.gpsimd.dma_start(
                out=csr[:], in_=s["CSR"].rearrange("(a b) -> b a", b=128)
            )
            s["rsum"], s["csr"] = rsum, csr

        def t_late(s):
            edig = stat.tile([128, mb_n], FP32, tag="edig", name="edig")
            nc.scalar.activation(edig[:], s["sdiag"][:], AF.Exp)
            negb = stat.tile([128, 2 * mb_n], FP32, tag="negb", name="negb")
            nc.vector.tensor_sub(negb[:, 0:mb_n], s["rsum"][:], edig[:])
            nc.vector.tensor_sub(negb[:, mb_n : 2 * mb_n], s["csr"][:], edig[:])
            lnb = stat.tile([128, 2 * mb_n], FP32, tag="lnb", name="lnb")
            nc.scalar.activation(lnb[:], negb[:], AF.Ln)
            res = stat.tile([128, mb_n], FP32, tag="res", name="res")
            nc.vector.tensor_add(res[:], lnb[:, 0:mb_n], lnb[:, mb_n : 2 * mb_n])
            d2 = stat.tile([128, mb_n], FP32, tag="d2", name="d2")
            nc.vector.tensor_scalar_mul(d2[:], s["sdiag"][:], -2.0)
            nc.vector.tensor_add(res[:], res[:], d2[:])
            nc.gpsimd.dma_start(
                out=OUT.rearrange("(a b) c -> b (a c)", b=128), in_=res[:]
            )

        # ---- pipelined driver ----
        states = {0: alloc_state()}
        p_loads_norms(states[0])
        p_inv(states[0])
        p_diag(states[0])
        for r in range(repeat):
            if r + 1 < repeat:
                states[r + 1] = alloc_state()
                p_loads_norms(states[r + 1])
            if r > 0:
                t_early(states[r - 1])
            b_phase(states[r], states.get(r + 1))
            if r > 0:
                t_late(states[r - 1])
                del states[r - 1]
        t_early(states[repeat - 1])
        t_late(states[repeat - 1])

    nc.compile()
    return nc


_CACHE = {}


def _get_nc():
    if "nc" not in _CACHE:
        _CACHE["nc"] = build()
    return _CACHE["nc"]


def make_in_maps(X, Y, ncores=NCORES, rpc=RPC):
    bf16 = ml_dtypes.bfloat16
    fp8 = ml_dtypes.float8_e4m3
    yt = np.ascontiguousarray(Y.T.astype(fp8))
    maps = []
    for i in range(ncores):
        xs = X[i * rpc : (i + 1) * rpc]
        maps.append({
            "XT": np.ascontiguousarray(xs.T.astype(bf16)),
            "XS": np.ascontiguousarray(xs.astype(fp8)),
            "YS": np.ascontiguousarray(Y[i * rpc : (i + 1) * rpc].astype(fp8)),
            "YT": yt,
        })
    return maps


def kernel(X, Y):
    X = np.ascontiguousarray(np.asarray(X, dtype=np.float32))
    Y = np.ascontiguousarray(np.asarray(Y, dtype=np.float32))
    assert X.shape == (BS, H) and Y.shape == (BS, H)
    nc = _get_nc()
    r = run_bass_kernel_spmd(nc, make_in_maps(X, Y), list(range(NCORES)))
    out = np.concatenate([r.results[i]["OUT"] for i in range(NCORES)], axis=0)
    return out.astype(np.float32)
